# revision 1
# baseline (speedup 1.0000x reference)
"""BertAttention (B=8, S=1024, H=1024, 16 heads) on 8 TRN2 NeuronCores.

Strategy: data-parallel over batch -- core b computes batch element b
end-to-end (QKV proj, attention, output proj, residual, LayerNorm).
No collectives needed.

Layout notes (per core):
  - All matmul contractions put the contracted dim on SBUF partitions.
  - Host pre-transposes X -> XT [H, S] and weights W -> W^T [in, out]
    so no on-device transposes are needed.
  - Scores are computed transposed: scT[k, q] = K_h Q_h^T so softmax
    denominators come from a ones-column folded into V (PV matmul row 64),
    and the PV matmul consumes exp tiles directly as the moving operand.
  - LayerNorm runs on the natural [q, o] output layout (free-dim reduce).
Matmul dtype is configurable: f32 / f32r (fp32 data, 1.5x PE rate) / bf16.
"""

import sys

sys.path.insert(0, "/opt/trn_rl_repo")

import numpy as np

B, S, H = 8, 1024, 1024
NH, HD = 16, 64
LN_EPS = 1e-12
N_CORES = 8

MM_DTYPE = "f32r"  # "f32" | "f32r" | "bf16"

_compiled = {}


def _build(mm_dtype, n_reps=1, use_gb=True):
    import concourse.tile as tile
    from concourse import bacc, mybir

    F32 = mybir.dt.float32
    AF = mybir.ActivationFunctionType
    ALU = mybir.AluOpType

    if mm_dtype == "f32":
        DT = F32
        DRAM_DT = F32
    elif mm_dtype == "f32r":
        DT = mybir.dt.float32r
        DRAM_DT = F32  # declare f32, bitcast APs at DMA time
    elif mm_dtype == "bf16":
        DT = mybir.dt.bfloat16
        DRAM_DT = mybir.dt.bfloat16
    else:
        raise ValueError(mm_dtype)

    def dma_in(out_ap, in_ap, eng=None):
        # DMA into a DT-typed tile; for f32r the DRAM side is f32 and we
        # bitcast the source AP (value-preserving; verified on HW).
        if eng is None:
            eng = nc.sync
        if mm_dtype == "f32r":
            in_ap = in_ap.bitcast(DT)
        eng.dma_start(out=out_ap, in_=in_ap)

    nc = bacc.Bacc("TRN2", target_bir_lowering=False)

    xt_d = nc.dram_tensor("xt", [H, S], DRAM_DT, kind="ExternalInput")
    xr_d = nc.dram_tensor("xr", [S, H], F32, kind="ExternalInput")
    wq_d = nc.dram_tensor("wq", [H, H], DRAM_DT, kind="ExternalInput")
    wk_d = nc.dram_tensor("wk", [H, H], DRAM_DT, kind="ExternalInput")
    wv_d = nc.dram_tensor("wv", [H, H], DRAM_DT, kind="ExternalInput")
    wo_d = nc.dram_tensor("wo", [H, H], DRAM_DT, kind="ExternalInput")
    bq_d = nc.dram_tensor("bq", [128, 8], F32, kind="ExternalInput")
    bk_d = nc.dram_tensor("bk", [128, 8], F32, kind="ExternalInput")
    bv_d = nc.dram_tensor("bv", [1, H], F32, kind="ExternalInput")
    mask_d = nc.dram_tensor("mask", [128, 8], F32, kind="ExternalInput")
    gamma_d = nc.dram_tensor("gamma", [1, H], F32, kind="ExternalInput")
    beta_d = nc.dram_tensor("beta", [1, H], F32, kind="ExternalInput")
    out_d = nc.dram_tensor("out", [S, H], F32, kind="ExternalOutput")

    NT = 8          # 128-row tiles per 1024 dim
    NCH = 2         # 512-col chunks per 1024 dim
    CH = 512

    with tile.TileContext(nc) as tc:
      for _rep in range(n_reps):
        with (
            tc.tile_pool(name="consts", bufs=1) as cp,
            tc.tile_pool(name="qt", bufs=8) as qt_pool,
            tc.tile_pool(name="kt", bufs=8) as kt_pool,
            tc.tile_pool(name="vt", bufs=8) as vt_pool,
        ):
            bq_sb = cp.tile([128, 8], F32)
            bk_sb = cp.tile([128, 8], F32)
            mask_sb = cp.tile([128, 8], F32)
            nc.sync.dma_start(out=bq_sb, in_=bq_d[:])
            nc.sync.dma_start(out=bk_sb, in_=bk_d[:])
            nc.sync.dma_start(out=mask_sb, in_=mask_d[:])
            bv_row = cp.tile([1, H], F32)
            gamma_row = cp.tile([1, H], F32)
            beta_row = cp.tile([1, H], F32)
            nc.sync.dma_start(out=bv_row, in_=bv_d[:])
            nc.sync.dma_start(out=gamma_row, in_=gamma_d[:])
            nc.sync.dma_start(out=beta_row, in_=beta_d[:])
            bv_sb = cp.tile([128, H], F32)
            gamma_sb = cp.tile([128, H], F32)
            beta_sb = cp.tile([128, H], F32)
            nc.gpsimd.partition_broadcast(bv_sb[:], bv_row[:])
            nc.gpsimd.partition_broadcast(gamma_sb[:], gamma_row[:])
            nc.gpsimd.partition_broadcast(beta_sb[:], beta_row[:])
            eps_sb = cp.tile([128, 1], F32)
            nc.vector.memset(eps_sb[:], LN_EPS)
            ones_sb = cp.tile([128, NH], F32)
            nc.vector.memset(ones_sb[:], 1.0)

            qt = [qt_pool.tile([128, S], DT, tag="qt", name=f"qt{t}") for t in range(NT)]
            kt = [kt_pool.tile([128, S], DT, tag="kt", name=f"kt{t}") for t in range(NT)]
            # v tiles: per k-tile, 16 heads x (64 v-cols + ones col)
            vt = [vt_pool.tile([128, NH * 65], DT, tag="vt", name=f"vt{t}") for t in range(NT)]

            # ---------------- QKV projections ----------------
            with (
                tc.tile_pool(name="xt", bufs=8) as xt_pool,
                tc.tile_pool(name="wp", bufs=13) as wp,
                tc.tile_pool(name="pp", bufs=4, space="PSUM") as pp,
            ):
                xt = []
                for t in range(NT):
                    x_t = xt_pool.tile([128, S], DT, tag="xt", name=f"xt{t}")
                    dma_in(x_t, xt_d[t * 128:(t + 1) * 128, :],
                           eng=(nc.scalar if t % 2 == 0 else nc.gpsimd))
                    xt.append(x_t)

                # V projection: natural [k, dv] layout; lhsT = XT tiles.
                wv_tiles = []
                for t in range(NT):
                    w_t = wp.tile([128, H], DT, tag="w", name=f"w_v{t}")
                    dma_in(w_t, wv_d[t * 128:(t + 1) * 128, :],
                           eng=(nc.sync if t % 2 == 0 else nc.scalar))
                    wv_tiles.append(w_t)
                for mk in range(NT):
                    # ones columns for the softmax-denominator rows
                    nc.vector.tensor_copy(
                        vt[mk][:].rearrange("p (g e) -> p g e", e=65)[:, :, 64:65],
                        ones_sb[:].rearrange("p (g e) -> p g e", e=1),
                    )
                    for n in range(NCH):
                        ps = pp.tile([128, CH], F32, tag="pp", name="pp_t")
                        for h in range(NT):
                            nc.tensor.matmul(
                                ps[:],
                                lhsT=xt[h][:, mk * 128:(mk + 1) * 128],
                                rhs=wv_tiles[h][:, n * CH:(n + 1) * CH],
                                start=(h == 0),
                                stop=(h == NT - 1),
                            )
                        nc.vector.tensor_add(
                            vt[mk][:, n * 8 * 65:(n + 1) * 8 * 65]
                            .rearrange("p (g e) -> p g e", e=65)[:, :, 0:64],
                            ps[:].rearrange("p (g e) -> p g e", e=64),
                            bv_sb[:, n * CH:(n + 1) * CH]
                            .rearrange("p (g e) -> p g e", e=64),
                        )

                for name, w_dram, b_sb, dst in (
                    ("q", wq_d, bq_sb, qt),
                    ("k", wk_d, bk_sb, kt),
                ):
                    w_tiles = []
                    for t in range(NT):
                        w_t = wp.tile([128, H], DT, tag="w", name=f"w_{name}{t}")
                        dma_in(w_t, w_dram[t * 128:(t + 1) * 128, :])
                        w_tiles.append(w_t)
                    for m in range(NT):
                        for n in range(NCH):
                            ps = pp.tile([128, CH], F32, tag="pp", name="pp_t")
                            for h in range(NT):
                                nc.tensor.matmul(
                                    ps[:],
                                    lhsT=w_tiles[h][:, m * 128:(m + 1) * 128],
                                    rhs=xt[h][:, n * CH:(n + 1) * CH],
                                    start=(h == 0),
                                    stop=(h == NT - 1),
                                )
                            nc.vector.tensor_scalar_add(
                                dst[m][:, n * CH:(n + 1) * CH], ps[:],
                                b_sb[:, m:m + 1],
                            )

            # ---------------- attention (per head pair) ----------------
            with (
                tc.tile_pool(name="ep", bufs=12) as ep,
                tc.tile_pool(name="rp", bufs=4) as rp,
                tc.tile_pool(name="rbp", bufs=3) as rbp,
                tc.tile_pool(name="scps", bufs=2, space="PSUM") as scps,
                tc.tile_pool(name="cxps", bufs=4, space="PSUM") as cxps,
            ):
                ctxt = []
                for t in range(NT):  # head pair t = heads 2t, 2t+1
                    ctx_t = qt_pool.tile([128, S], DT, tag="qt", name=f"ctx{t}")
                    ctxt.append(ctx_t)
                    # 4 live PV accumulators: (head, chunk)
                    cxs = [[cxps.tile([65, CH], F32, tag="cx", name="cx_t")
                            for _ in range(NCH)] for _ in range(2)]
                    for k in range(NT):
                        # one [128, S] score psum per head per k-tile (2 banks);
                        # the two q-chunks fill its halves; one exp covers both.
                        # Head A (rows 0-63) and head B (rows 64-127) matmuls are
                        # emitted adjacently per chunk: disjoint PE row groups run
                        # concurrently (row tiling).
                        scs = []
                        for hh in range(2):
                            sc = scps.tile([128, S], F32, tag="sc", name="sc_t")
                            scs.append(sc)
                        for n in range(NCH):
                            for hh in range(2):
                                p0 = hh * 64
                                nc.tensor.matmul(
                                    scs[hh][:, n * CH:(n + 1) * CH],
                                    lhsT=kt[t][p0:p0 + 64, k * 128:(k + 1) * 128],
                                    rhs=qt[t][p0:p0 + 64, n * CH:(n + 1) * CH],
                                    start=True,
                                    stop=True,
                                )
                        for hh in range(2):
                            g = 2 * t + hh
                            e_t = ep.tile([128, S], DT, tag="e", name="e_t")
                            nc.scalar.activation(
                                e_t[:], scs[hh][:], AF.Exp,
                                bias=mask_sb[:, k:k + 1], scale=0.125,
                            )
                            for n in range(NCH):
                                nc.tensor.matmul(
                                    cxs[hh][n][:],
                                    lhsT=vt[k][:, g * 65:(g + 1) * 65],
                                    rhs=e_t[:, n * CH:(n + 1) * CH],
                                    start=(k == 0),
                                    stop=(k == NT - 1),
                                )
                    for hh in range(2):
                        for n in range(NCH):
                            cx = cxs[hh][n]
                            recip = rp.tile([1, CH], F32, tag="recip", name="recip_t")
                            nc.vector.reciprocal(recip[:], cx[64:65, :])
                            rb = rbp.tile([64, CH], F32, tag="rb", name="rb_t")
                            nc.gpsimd.partition_broadcast(rb[:], recip[:])
                            nc.vector.tensor_mul(
                                ctx_t[hh * 64:hh * 64 + 64, n * CH:(n + 1) * CH],
                                cx[0:64, :],
                                rb[:],
                            )

            # ---------------- output proj + residual + LayerNorm ----------------
            with (
                tc.tile_pool(name="wo", bufs=8) as wop,
                tc.tile_pool(name="xr", bufs=5) as xrp,
                tc.tile_pool(name="ob", bufs=4) as obp,
                tc.tile_pool(name="st", bufs=4) as stp,
                tc.tile_pool(name="po", bufs=4, space="PSUM") as po,
            ):
                wo_tiles = []
                for t in range(NT):
                    w_t = wop.tile([128, H], DT, tag="wo", name=f"wo{t}")
                    dma_in(w_t, wo_d[t * 128:(t + 1) * 128, :])
                    wo_tiles.append(w_t)
                for mq in range(NT):
                    xr_t = xrp.tile([128, H], F32, tag="xr", name="xr_t")
                    (nc.sync if mq % 2 == 0 else nc.gpsimd).dma_start(
                        out=xr_t, in_=xr_d[mq * 128:(mq + 1) * 128, :]
                    )
                    o_t = obp.tile([128, H], F32, tag="ob", name="ob_t")
                    for n in range(NCH):
                        ps = po.tile([128, CH], F32, tag="po", name="po_t")
                        for t in range(NT):
                            nc.tensor.matmul(
                                ps[:],
                                lhsT=ctxt[t][:, mq * 128:(mq + 1) * 128],
                                rhs=wo_tiles[t][:, n * CH:(n + 1) * CH],
                                start=(t == 0),
                                stop=(t == NT - 1),
                            )
                        nc.vector.tensor_add(
                            o_t[:, n * CH:(n + 1) * CH], ps[:],
                            xr_t[:, n * CH:(n + 1) * CH],
                        )
                    stats = stp.tile([128, 2, 6], F32, tag="stats", name="stats_t")
                    for sg in range(2):
                        nc.vector.bn_stats(
                            stats[:, sg, :], o_t[:, sg * CH:(sg + 1) * CH]
                        )
                    mv = stp.tile([128, 2], F32, tag="mv", name="mv_t")
                    nc.vector.bn_aggr(mv[:], stats[:])
                    mu = mv[:, 0:1]
                    var = mv[:, 1:2]
                    std = stp.tile([128, 1], F32, tag="std", name="std_t")
                    nc.scalar.activation(std[:], var[:], AF.Sqrt, bias=eps_sb[:])
                    rstd = stp.tile([128, 1], F32, tag="rstd", name="rstd_t")
                    nc.vector.reciprocal(rstd[:], std[:])
                    # (x - mu) * rstd as ACT affine: rstd*x + (-mu*rstd)
                    nmur = stp.tile([128, 1], F32, tag="nmur", name="nmur_t")
                    nc.vector.tensor_scalar(
                        out=nmur[:], in0=mu, scalar1=rstd[:], scalar2=-1.0,
                        op0=ALU.mult, op1=ALU.mult,
                    )
                    nc.scalar.activation(
                        o_t[:], o_t[:], AF.Identity,
                        bias=nmur[:], scale=rstd[:],
                    )
                    if use_gb:
                        nc.vector.tensor_mul(o_t[:], o_t[:], gamma_sb[:])
                        nc.vector.tensor_add(o_t[:], o_t[:], beta_sb[:])
                    (nc.gpsimd if mq % 2 == 0 else nc.sync).dma_start(
                        out=out_d[mq * 128:(mq + 1) * 128, :], in_=o_t
                    )

    nc.compile()
    return nc


def _host_prep(mm_dtype, hidden_states, attention_mask, Wq, bq, Wk, bk, Wv, bv,
               Wo, bo, ln_gamma, ln_beta):
    f32 = np.float32
    hs = np.ascontiguousarray(hidden_states, dtype=f32)
    if mm_dtype == "bf16":
        import ml_dtypes
        wdt = ml_dtypes.bfloat16
    else:
        wdt = f32
    wqT = np.ascontiguousarray(np.asarray(Wq, dtype=f32).T).astype(wdt)
    wkT = np.ascontiguousarray(np.asarray(Wk, dtype=f32).T).astype(wdt)
    wvT = np.ascontiguousarray(np.asarray(Wv, dtype=f32).T).astype(wdt)
    woT = np.ascontiguousarray(np.asarray(Wo, dtype=f32).T).astype(wdt)
    bq_r = np.ascontiguousarray(np.asarray(bq, f32).reshape(8, 128).T)
    bk_r = np.ascontiguousarray(np.asarray(bk, f32).reshape(8, 128).T)
    bv_r = np.ascontiguousarray(np.asarray(bv, f32).reshape(1, H))
    gamma_r = np.ascontiguousarray(np.asarray(ln_gamma, f32).reshape(1, H))
    beta_r = np.ascontiguousarray(np.asarray(ln_beta, f32).reshape(1, H))
    bo_r = np.asarray(bo, f32)
    mask = np.asarray(attention_mask, f32).reshape(B, S)

    in_maps = []
    for b in range(B):
        xt = np.ascontiguousarray(hs[b].T).astype(wdt)
        xr = np.ascontiguousarray(hs[b] + bo_r[None, :])
        mask_r = np.ascontiguousarray(mask[b].reshape(8, 128).T)
        in_maps.append({
            "xt": xt, "xr": xr,
            "wq": wqT, "wk": wkT, "wv": wvT, "wo": woT,
            "bq": bq_r, "bk": bk_r, "bv": bv_r,
            "mask": mask_r, "gamma": gamma_r, "beta": beta_r,
        })
    return in_maps


def get_nc(mm_dtype=MM_DTYPE, n_reps=1, use_gb=True):
    key = (mm_dtype, n_reps, use_gb)
    if key not in _compiled:
        _compiled[key] = _build(mm_dtype, n_reps, use_gb)
    return _compiled[key]


def kernel(hidden_states, attention_mask, Wq, bq, Wk, bk, Wv, bv, Wo, bo,
           ln_gamma, ln_beta):
    from concourse.bass_utils import run_bass_kernel_spmd

    use_gb = not (
        np.all(np.asarray(ln_gamma) == 1.0) and np.all(np.asarray(ln_beta) == 0.0)
    )
    nc = get_nc(MM_DTYPE, use_gb=use_gb)
    in_maps = _host_prep(MM_DTYPE, hidden_states, attention_mask, Wq, bq,
                         Wk, bk, Wv, bv, Wo, bo, ln_gamma, ln_beta)
    res = run_bass_kernel_spmd(nc, in_maps, list(range(N_CORES)))
    out = np.stack([np.asarray(res.results[i]["out"]) for i in range(N_CORES)])
    return out.astype(np.float32)



# revision 2
# speedup vs baseline: 1.8343x; 1.8343x over previous
"""BertAttention (B=8, S=1024, H=1024, 16 heads) on 8 TRN2 NeuronCores.

Strategy: data-parallel over batch -- core b computes batch element b
end-to-end (QKV proj, attention, output proj, residual, LayerNorm).
No collectives needed.

Layout notes (per core):
  - All matmul contractions put the contracted dim on SBUF partitions.
  - Host pre-transposes X -> XT [H, S] and weights W -> W^T [in, out]
    so no on-device transposes are needed.
  - Scores are computed transposed: scT[k, q] = K_h Q_h^T so softmax
    denominators come from a ones-column folded into V (PV matmul row 64),
    and the PV matmul consumes exp tiles directly as the moving operand.
  - LayerNorm runs on the natural [q, o] output layout (free-dim reduce).
Matmul dtype is configurable: f32 / f32r (fp32 data, 1.5x PE rate) / bf16.
"""

import sys

sys.path.insert(0, "/opt/trn_rl_repo")

import numpy as np

B, S, H = 8, 1024, 1024
NH, HD = 16, 64
LN_EPS = 1e-12
N_CORES = 8

MM_DTYPE = "bf16"  # "f32" | "f32r" | "bf16"

_compiled = {}


def _build(mm_dtype, n_reps=1, use_gb=True):
    import concourse.tile as tile
    from concourse import bacc, mybir

    F32 = mybir.dt.float32
    AF = mybir.ActivationFunctionType
    ALU = mybir.AluOpType

    if mm_dtype == "f32":
        DT = F32
        DRAM_DT = F32
    elif mm_dtype == "f32r":
        DT = mybir.dt.float32r
        DRAM_DT = F32  # declare f32, bitcast APs at DMA time
    elif mm_dtype == "bf16":
        DT = mybir.dt.bfloat16
        DRAM_DT = mybir.dt.bfloat16
    else:
        raise ValueError(mm_dtype)

    def dma_in(out_ap, in_ap, eng=None):
        # DMA into a DT-typed tile; for f32r the DRAM side is f32 and we
        # bitcast the source AP (value-preserving; verified on HW).
        if eng is None:
            eng = nc.sync
        if mm_dtype == "f32r":
            in_ap = in_ap.bitcast(DT)
        eng.dma_start(out=out_ap, in_=in_ap)

    nc = bacc.Bacc("TRN2", target_bir_lowering=False)

    xt_d = nc.dram_tensor("xt", [H, S], DRAM_DT, kind="ExternalInput")
    xr_d = nc.dram_tensor("xr", [S, H], F32, kind="ExternalInput")
    wq_d = nc.dram_tensor("wq", [H, H], DRAM_DT, kind="ExternalInput")
    wk_d = nc.dram_tensor("wk", [H, H], DRAM_DT, kind="ExternalInput")
    wv_d = nc.dram_tensor("wv", [H, H], DRAM_DT, kind="ExternalInput")
    wo_d = nc.dram_tensor("wo", [H, H], DRAM_DT, kind="ExternalInput")
    bq_d = nc.dram_tensor("bq", [128, 8], F32, kind="ExternalInput")
    bk_d = nc.dram_tensor("bk", [128, 8], F32, kind="ExternalInput")
    bv_d = nc.dram_tensor("bv", [1, H], F32, kind="ExternalInput")
    mask_d = nc.dram_tensor("mask", [128, 8], F32, kind="ExternalInput")
    gamma_d = nc.dram_tensor("gamma", [1, H], F32, kind="ExternalInput")
    beta_d = nc.dram_tensor("beta", [1, H], F32, kind="ExternalInput")
    out_d = nc.dram_tensor("out", [S, H], F32, kind="ExternalOutput")

    NT = 8          # 128-row tiles per 1024 dim
    NCH = 2         # 512-col chunks per 1024 dim
    CH = 512

    with tile.TileContext(nc) as tc:
      for _rep in range(n_reps):
        with (
            tc.tile_pool(name="consts", bufs=1) as cp,
            tc.tile_pool(name="qt", bufs=8) as qt_pool,
            tc.tile_pool(name="kt", bufs=8) as kt_pool,
            tc.tile_pool(name="vt", bufs=8) as vt_pool,
        ):
            bq_sb = cp.tile([128, 8], F32)
            bk_sb = cp.tile([128, 8], F32)
            mask_sb = cp.tile([128, 8], F32)
            nc.sync.dma_start(out=bq_sb, in_=bq_d[:])
            nc.sync.dma_start(out=bk_sb, in_=bk_d[:])
            nc.sync.dma_start(out=mask_sb, in_=mask_d[:])
            bv_row = cp.tile([1, H], F32)
            gamma_row = cp.tile([1, H], F32)
            beta_row = cp.tile([1, H], F32)
            nc.sync.dma_start(out=bv_row, in_=bv_d[:])
            nc.sync.dma_start(out=gamma_row, in_=gamma_d[:])
            nc.sync.dma_start(out=beta_row, in_=beta_d[:])
            bv_sb = cp.tile([128, H], F32)
            gamma_sb = cp.tile([128, H], F32)
            beta_sb = cp.tile([128, H], F32)
            nc.gpsimd.partition_broadcast(bv_sb[:], bv_row[:])
            nc.gpsimd.partition_broadcast(gamma_sb[:], gamma_row[:])
            nc.gpsimd.partition_broadcast(beta_sb[:], beta_row[:])
            eps_sb = cp.tile([128, 1], F32)
            nc.vector.memset(eps_sb[:], LN_EPS)
            ones_sb = cp.tile([128, NH], F32)
            nc.vector.memset(ones_sb[:], 1.0)

            qt = [qt_pool.tile([128, S], DT, tag="qt", name=f"qt{t}") for t in range(NT)]
            kt = [kt_pool.tile([128, S], DT, tag="kt", name=f"kt{t}") for t in range(NT)]
            # v tiles: per k-tile, 16 heads x (64 v-cols + ones col)
            vt = [vt_pool.tile([128, NH * 65], DT, tag="vt", name=f"vt{t}") for t in range(NT)]

            # ---------------- QKV projections ----------------
            with (
                tc.tile_pool(name="xt", bufs=8) as xt_pool,
                tc.tile_pool(name="wp", bufs=13) as wp,
                tc.tile_pool(name="pp", bufs=4, space="PSUM") as pp,
            ):
                xt = []
                for t in range(NT):
                    x_t = xt_pool.tile([128, S], DT, tag="xt", name=f"xt{t}")
                    dma_in(x_t, xt_d[t * 128:(t + 1) * 128, :],
                           eng=(nc.scalar if t % 2 == 0 else nc.gpsimd))
                    xt.append(x_t)

                # V projection: natural [k, dv] layout; lhsT = XT tiles.
                wv_tiles = []
                for t in range(NT):
                    w_t = wp.tile([128, H], DT, tag="w", name=f"w_v{t}")
                    dma_in(w_t, wv_d[t * 128:(t + 1) * 128, :],
                           eng=(nc.sync if t % 2 == 0 else nc.scalar))
                    wv_tiles.append(w_t)
                for mk in range(NT):
                    # ones columns for the softmax-denominator rows
                    nc.vector.tensor_copy(
                        vt[mk][:].rearrange("p (g e) -> p g e", e=65)[:, :, 64:65],
                        ones_sb[:].rearrange("p (g e) -> p g e", e=1),
                    )
                    for n in range(NCH):
                        ps = pp.tile([128, CH], F32, tag="pp", name="pp_t")
                        for h in range(NT):
                            nc.tensor.matmul(
                                ps[:],
                                lhsT=xt[h][:, mk * 128:(mk + 1) * 128],
                                rhs=wv_tiles[h][:, n * CH:(n + 1) * CH],
                                start=(h == 0),
                                stop=(h == NT - 1),
                            )
                        nc.vector.tensor_add(
                            vt[mk][:, n * 8 * 65:(n + 1) * 8 * 65]
                            .rearrange("p (g e) -> p g e", e=65)[:, :, 0:64],
                            ps[:].rearrange("p (g e) -> p g e", e=64),
                            bv_sb[:, n * CH:(n + 1) * CH]
                            .rearrange("p (g e) -> p g e", e=64),
                        )

                for name, w_dram, b_sb, dst in (
                    ("q", wq_d, bq_sb, qt),
                    ("k", wk_d, bk_sb, kt),
                ):
                    w_tiles = []
                    for t in range(NT):
                        w_t = wp.tile([128, H], DT, tag="w", name=f"w_{name}{t}")
                        dma_in(w_t, w_dram[t * 128:(t + 1) * 128, :])
                        w_tiles.append(w_t)
                    for m in range(NT):
                        for n in range(NCH):
                            ps = pp.tile([128, CH], F32, tag="pp", name="pp_t")
                            for h in range(NT):
                                nc.tensor.matmul(
                                    ps[:],
                                    lhsT=w_tiles[h][:, m * 128:(m + 1) * 128],
                                    rhs=xt[h][:, n * CH:(n + 1) * CH],
                                    start=(h == 0),
                                    stop=(h == NT - 1),
                                )
                            nc.vector.tensor_scalar_add(
                                dst[m][:, n * CH:(n + 1) * CH], ps[:],
                                b_sb[:, m:m + 1],
                            )

            # ---------------- attention (per head pair) ----------------
            with (
                tc.tile_pool(name="ep", bufs=12) as ep,
                tc.tile_pool(name="rp", bufs=4) as rp,
                tc.tile_pool(name="rbp", bufs=3) as rbp,
                tc.tile_pool(name="scps", bufs=2, space="PSUM") as scps,
                tc.tile_pool(name="cxps", bufs=4, space="PSUM") as cxps,
            ):
                ctxt = []
                for t in range(NT):  # head pair t = heads 2t, 2t+1
                    ctx_t = qt_pool.tile([128, S], DT, tag="qt", name=f"ctx{t}")
                    ctxt.append(ctx_t)
                    # 4 live PV accumulators: (head, chunk)
                    cxs = [[cxps.tile([65, CH], F32, tag="cx", name="cx_t")
                            for _ in range(NCH)] for _ in range(2)]
                    for k in range(NT):
                        # one [128, S] score psum per head per k-tile (2 banks);
                        # the two q-chunks fill its halves; one exp covers both.
                        # Head A (rows 0-63) and head B (rows 64-127) matmuls are
                        # emitted adjacently per chunk: disjoint PE row groups run
                        # concurrently (row tiling).
                        scs = []
                        for hh in range(2):
                            sc = scps.tile([128, S], F32, tag="sc", name="sc_t")
                            scs.append(sc)
                        for n in range(NCH):
                            for hh in range(2):
                                p0 = hh * 64
                                nc.tensor.matmul(
                                    scs[hh][:, n * CH:(n + 1) * CH],
                                    lhsT=kt[t][p0:p0 + 64, k * 128:(k + 1) * 128],
                                    rhs=qt[t][p0:p0 + 64, n * CH:(n + 1) * CH],
                                    start=True,
                                    stop=True,
                                )
                        for hh in range(2):
                            g = 2 * t + hh
                            e_t = ep.tile([128, S], DT, tag="e", name="e_t")
                            nc.scalar.activation(
                                e_t[:], scs[hh][:], AF.Exp,
                                bias=mask_sb[:, k:k + 1], scale=0.125,
                            )
                            for n in range(NCH):
                                nc.tensor.matmul(
                                    cxs[hh][n][:],
                                    lhsT=vt[k][:, g * 65:(g + 1) * 65],
                                    rhs=e_t[:, n * CH:(n + 1) * CH],
                                    start=(k == 0),
                                    stop=(k == NT - 1),
                                )
                    for hh in range(2):
                        for n in range(NCH):
                            cx = cxs[hh][n]
                            recip = rp.tile([1, CH], F32, tag="recip", name="recip_t")
                            nc.vector.reciprocal(recip[:], cx[64:65, :])
                            rb = rbp.tile([64, CH], F32, tag="rb", name="rb_t")
                            nc.gpsimd.partition_broadcast(rb[:], recip[:])
                            nc.vector.tensor_mul(
                                ctx_t[hh * 64:hh * 64 + 64, n * CH:(n + 1) * CH],
                                cx[0:64, :],
                                rb[:],
                            )

            # ---------------- output proj + residual + LayerNorm ----------------
            with (
                tc.tile_pool(name="wo", bufs=8) as wop,
                tc.tile_pool(name="xr", bufs=5) as xrp,
                tc.tile_pool(name="ob", bufs=4) as obp,
                tc.tile_pool(name="st", bufs=4) as stp,
                tc.tile_pool(name="po", bufs=4, space="PSUM") as po,
            ):
                wo_tiles = []
                for t in range(NT):
                    w_t = wop.tile([128, H], DT, tag="wo", name=f"wo{t}")
                    dma_in(w_t, wo_d[t * 128:(t + 1) * 128, :])
                    wo_tiles.append(w_t)
                for mq in range(NT):
                    xr_t = xrp.tile([128, H], F32, tag="xr", name="xr_t")
                    (nc.sync if mq % 2 == 0 else nc.gpsimd).dma_start(
                        out=xr_t, in_=xr_d[mq * 128:(mq + 1) * 128, :]
                    )
                    o_t = obp.tile([128, H], F32, tag="ob", name="ob_t")
                    for n in range(NCH):
                        ps = po.tile([128, CH], F32, tag="po", name="po_t")
                        for t in range(NT):
                            nc.tensor.matmul(
                                ps[:],
                                lhsT=ctxt[t][:, mq * 128:(mq + 1) * 128],
                                rhs=wo_tiles[t][:, n * CH:(n + 1) * CH],
                                start=(t == 0),
                                stop=(t == NT - 1),
                            )
                        nc.vector.tensor_add(
                            o_t[:, n * CH:(n + 1) * CH], ps[:],
                            xr_t[:, n * CH:(n + 1) * CH],
                        )
                    stats = stp.tile([128, 2, 6], F32, tag="stats", name="stats_t")
                    for sg in range(2):
                        nc.vector.bn_stats(
                            stats[:, sg, :], o_t[:, sg * CH:(sg + 1) * CH]
                        )
                    mv = stp.tile([128, 2], F32, tag="mv", name="mv_t")
                    nc.vector.bn_aggr(mv[:], stats[:])
                    mu = mv[:, 0:1]
                    var = mv[:, 1:2]
                    std = stp.tile([128, 1], F32, tag="std", name="std_t")
                    nc.scalar.activation(std[:], var[:], AF.Sqrt, bias=eps_sb[:])
                    rstd = stp.tile([128, 1], F32, tag="rstd", name="rstd_t")
                    nc.vector.reciprocal(rstd[:], std[:])
                    # (x - mu) * rstd as ACT affine: rstd*x + (-mu*rstd)
                    nmur = stp.tile([128, 1], F32, tag="nmur", name="nmur_t")
                    nc.vector.tensor_scalar(
                        out=nmur[:], in0=mu, scalar1=rstd[:], scalar2=-1.0,
                        op0=ALU.mult, op1=ALU.mult,
                    )
                    nc.scalar.activation(
                        o_t[:], o_t[:], AF.Identity,
                        bias=nmur[:], scale=rstd[:],
                    )
                    if use_gb:
                        nc.vector.tensor_mul(o_t[:], o_t[:], gamma_sb[:])
                        nc.vector.tensor_add(o_t[:], o_t[:], beta_sb[:])
                    (nc.gpsimd if mq % 2 == 0 else nc.sync).dma_start(
                        out=out_d[mq * 128:(mq + 1) * 128, :], in_=o_t
                    )

    nc.compile()
    return nc


def _host_prep(mm_dtype, hidden_states, attention_mask, Wq, bq, Wk, bk, Wv, bv,
               Wo, bo, ln_gamma, ln_beta):
    f32 = np.float32
    hs = np.ascontiguousarray(hidden_states, dtype=f32)
    if mm_dtype == "bf16":
        import ml_dtypes
        wdt = ml_dtypes.bfloat16
    else:
        wdt = f32
    wqT = np.ascontiguousarray(np.asarray(Wq, dtype=f32).T).astype(wdt)
    wkT = np.ascontiguousarray(np.asarray(Wk, dtype=f32).T).astype(wdt)
    wvT = np.ascontiguousarray(np.asarray(Wv, dtype=f32).T).astype(wdt)
    woT = np.ascontiguousarray(np.asarray(Wo, dtype=f32).T).astype(wdt)
    bq_r = np.ascontiguousarray(np.asarray(bq, f32).reshape(8, 128).T)
    bk_r = np.ascontiguousarray(np.asarray(bk, f32).reshape(8, 128).T)
    bv_r = np.ascontiguousarray(np.asarray(bv, f32).reshape(1, H))
    gamma_r = np.ascontiguousarray(np.asarray(ln_gamma, f32).reshape(1, H))
    beta_r = np.ascontiguousarray(np.asarray(ln_beta, f32).reshape(1, H))
    bo_r = np.asarray(bo, f32)
    mask = np.asarray(attention_mask, f32).reshape(B, S)

    in_maps = []
    for b in range(B):
        xt = np.ascontiguousarray(hs[b].T).astype(wdt)
        xr = np.ascontiguousarray(hs[b] + bo_r[None, :])
        mask_r = np.ascontiguousarray(mask[b].reshape(8, 128).T)
        in_maps.append({
            "xt": xt, "xr": xr,
            "wq": wqT, "wk": wkT, "wv": wvT, "wo": woT,
            "bq": bq_r, "bk": bk_r, "bv": bv_r,
            "mask": mask_r, "gamma": gamma_r, "beta": beta_r,
        })
    return in_maps


def get_nc(mm_dtype=MM_DTYPE, n_reps=1, use_gb=True):
    key = (mm_dtype, n_reps, use_gb)
    if key not in _compiled:
        _compiled[key] = _build(mm_dtype, n_reps, use_gb)
    return _compiled[key]


def kernel(hidden_states, attention_mask, Wq, bq, Wk, bk, Wv, bv, Wo, bo,
           ln_gamma, ln_beta):
    from concourse.bass_utils import run_bass_kernel_spmd

    use_gb = not (
        np.all(np.asarray(ln_gamma) == 1.0) and np.all(np.asarray(ln_beta) == 0.0)
    )
    nc = get_nc(MM_DTYPE, use_gb=use_gb)
    in_maps = _host_prep(MM_DTYPE, hidden_states, attention_mask, Wq, bq,
                         Wk, bk, Wv, bv, Wo, bo, ln_gamma, ln_beta)
    res = run_bass_kernel_spmd(nc, in_maps, list(range(N_CORES)))
    out = np.stack([np.asarray(res.results[i]["out"]) for i in range(N_CORES)])
    return out.astype(np.float32)

